# revision 2
# baseline (speedup 1.0000x reference)
"""MoE conditional feed-forward (T=1024, D=1024, H=2048, E=32, K=2) on 8 trn2 cores.

Sharding: expert-parallel, E/8 = 4 experts per core. Host gathers the tokens
routed to each expert (dispatch), the device runs the expert FFNs on padded
token blocks, the host scatters results back (combine).

v2: weights are stored int8 in DRAM (per-channel symmetric quantization,
host-side) and dequantized to fp16 on-chip, halving the HBM weight traffic
that bound v1 (51 MB -> 26 MB per core). Dequant ops are split between the
vector (DVE, 2 elem/cyc) and scalar (ACT, 1 elem/cyc) engines so they hide
under the weight DMA. Quantization axes are chosen so each SBUF weight
slice has a constant scale per partition row:
  Wgu: scale per (e, d)            -> partition dim of the stationary tile
  Wd:  scale per (e, half, h%128)  -> partition dim of the streamed tile
Measured numpy end-to-end rel err of this scheme: 1.4e-2 (gate: 2e-2).

Device dataflow per (expert, token-block) work item ("feature-major", no
transposes):
  stage 1: gate/up = Wgu_tile.T @ xT   -> PSUM (h=128, C), 8 d-chunk
           accumulation; silu(gate) * up -> inter SBUF tile (h=128, C) fp16.
  stage 2: out = inter_chunk.T @ WdT   -> PSUM (C, 1024), 16 h-chunk
           accumulation.
All accumulation fp32 in PSUM; output stored fp16, upcast on host.
"""

import numpy as np

T, D, H, E, K = 1024, 1024, 2048, 32, 2
NCORES = 8
EPC = E // NCORES  # experts per core
C = 128            # token capacity per work item (one PE column block)

_CACHE: dict = {}


def _build(
    nw: int,
    cap: int = C,
    loop_n: int | None = None,
    probe_dma_only: bool = False,
    rep: int = 1,
    dve_dc: int = 5,
    wd_eng: str = "alt",
    probe: str = "",
    hp_epi: int = 0,
):
    """Build + compile the SPMD Bass program for nw work items per core.

    loop_n wraps the body in a hardware For_i loop (same work each
    iteration) for differential wall-clock timing in bench2.py.
    probe_dma_only emits only the DMA traffic (garbage outputs) to measure
    the memory floor. dve_dc: wgu dequant slices with dc < dve_dc go to the
    vector engine, the rest to the scalar engine. wd_eng: which engine
    dequantizes Wd ("dve" | "act" | "split").
    """
    import contextlib

    import concourse.bass as bass
    import concourse.mybir as mybir
    import concourse.tile as tile
    from concourse import bacc

    _nullctx = contextlib.nullcontext

    if probe_dma_only:
        probe = "dma"
    do_dequant = probe not in ("dma", "nodeq")
    do_compute = probe not in ("dma", "deqonly")

    i8 = mybir.dt.int8
    f16 = mybir.dt.float16
    f32 = mybir.dt.float32

    nc = bacc.Bacc(
        "TRN2",
        target_bir_lowering=False,
        debug=False,
        enable_asserts=False,
        num_devices=NCORES,
    )

    # Per-core DRAM parameters (host pre-arranged, partition-major):
    #   xt  : [nw, 128, 8, C]  fp16   xt[j, dp, dc, c] = x[tok_c, dc*128+dp]
    #   wgu8: [nw, 2, 128, 8, 2048] int8
    #         [half, dp, dc, gsub*1024 + (ht*2+g)*128 + hl]
    #           = q(Wgu)[e, g, (half*2+gsub)*512+ht*128+hl, dc*128+dp]
    #   wd8 : [nw, 2, 128, 2, 4, 1024] int8  [half, hl, gsub, i, d] =
    #         q(Wd)[e, d, ((half*2+gsub)*4+i)*128+hl]
    #   sc  : [nw, 128, 12] fp32 scales:
    #         sc[j, dp, dc]     = sgu[e, dc*128+dp]      (dc in 0..7)
    #         sc[j, hl, 8+half] = swd[e, half, hl]       (half in 0..1)
    #   out : [nw, C, 1024] fp16 (upcast on host)
    xt_d = nc.dram_tensor("xt", [nw, 128, 8, cap], f16, kind="ExternalInput").ap()
    wgu_d = nc.dram_tensor(
        "wgu8", [nw, 2, 128, 8, 2048], i8, kind="ExternalInput"
    ).ap()
    wd_d = nc.dram_tensor(
        "wd8", [nw, 2, 128, 8192], i8, kind="ExternalInput"
    ).ap()
    sc_d = nc.dram_tensor("sc", [nw, 128, 12], f32, kind="ExternalInput").ap()
    out_d = nc.dram_tensor("out", [nw, cap, 1024], f16, kind="ExternalOutput").ap()

    silu = mybir.ActivationFunctionType.Silu
    copy_f = mybir.ActivationFunctionType.Copy

    with tile.TileContext(nc) as tc:
        with (
            tc.tile_pool(name="wgu8_p", bufs=2) as wgu8_p,
            tc.tile_pool(name="wgu16d_p", bufs=2) as wgu16d_p,
            tc.tile_pool(name="wgu16a_p", bufs=2) as wgu16a_p,
            tc.tile_pool(name="wd8_p", bufs=2) as wd8_p,
            tc.tile_pool(name="wd16_p", bufs=2) as wd16_p,
            tc.tile_pool(name="sc_p", bufs=2) as sc_p,
            tc.tile_pool(name="xt_p", bufs=2) as xt_p,
            tc.tile_pool(name="inter_p", bufs=2) as inter_p,
            tc.tile_pool(name="sg_p", bufs=2) as sg_p,
            tc.tile_pool(name="o_p", bufs=2) as o_p,
            tc.tile_pool(name="ps_gu", bufs=2, space="PSUM") as ps_gu,
            tc.tile_pool(name="ps_dn", bufs=2, space="PSUM") as ps_dn,
        ):
            # Loads go on the sync (SP) HWDGE ring; stores on the scalar
            # (ACT) ring. A store waits on compute, and HWDGE rings are
            # FIFO — sharing one ring would head-of-line-block the next
            # expert's weight loads behind each output store.
            load_eng = nc.sync
            store_eng = nc.scalar

            def dma(out_ap, in_ap):
                load_eng.dma_start(out=out_ap, in_=in_ap)

            n_act = 8 - dve_dc

            def emit_body():
                # Software-pipelined over units u = (kind, j, half):
                # per item j the units are gu0, gu1, d0, d1. Emission order
                # L(u+2); Q(u+1); C(u) so each engine's queue carries the
                # dequant for the NEXT unit before the epilogue of the
                # current one, and DMA runs two units ahead.
                units = []
                for j in [jj for _ in range(rep) for jj in range(nw)]:
                    units += [("gu", j, 0), ("gu", j, 1), ("d", j, 0), ("d", j, 1)]
                st: dict = {}      # per-unit tiles
                it: dict = {}      # per-item tiles (keyed by position index)

                def load(ui):
                    kind, j, half = units[ui]
                    if kind == "gu" and half == 0:
                        xt_sb = xt_p.tile([128, 8, cap], f16)
                        dma(xt_sb[:], xt_d[j])
                        sc_sb = sc_p.tile([128, 12], f32)
                        dma(sc_sb[:], sc_d[j])
                        it[ui] = {"xt": xt_sb, "sc": sc_sb,
                                  "inter": inter_p.tile([128, 16, cap], f16, name="inter")}
                    if kind == "gu":
                        t8 = wgu8_p.tile([128, 8, 2048], i8)
                        dma(t8[:], wgu_d[j, half])
                    else:
                        t8 = wd8_p.tile([128, 8192], i8)
                        dma(t8[:], wd_d[j, half])
                    st[ui] = {"t8": t8}

                def item_of(ui):
                    # the per-item dict lives at the item's first unit index
                    return it[ui - ui % 4]

                def dequant_thunks(ui):
                    """Allocate the fp16 tiles for unit ui and return a list
                    of thunks, each emitting one dequant op. The driver
                    interleaves them between compute groups so neither the
                    epilogue nor the next unit's dequant head-of-line-blocks
                    the other on the DVE/ACT queues."""
                    kind, j, half = units[ui]
                    if probe == "dma":
                        return []
                    t8 = st[ui]["t8"]
                    sc_sb = item_of(ui)["sc"]
                    thunks = []
                    if kind == "gu":
                        td = wgu16d_p.tile([128, max(dve_dc, 1), 2048], f16)
                        ta = wgu16a_p.tile([128, max(n_act, 1), 2048], f16)
                        for dc in range(8):
                            s1 = sc_sb[:, dc : dc + 1]
                            if dc < dve_dc:
                                dst, src = td[:, dc], t8[:, dc]
                                if not do_dequant:
                                    dst, src = td[:, dc, ::32], t8[:, dc, ::32]
                                thunks.append(
                                    lambda d=dst, s=src, sc=s1:
                                    nc.vector.tensor_scalar_mul(d, s, sc)
                                )
                            else:
                                dst, src = ta[:, dc - dve_dc], t8[:, dc]
                                if not do_dequant:
                                    dst, src = ta[:, dc - dve_dc, ::32], t8[:, dc, ::32]
                                thunks.append(
                                    lambda d=dst, s=src, sc=s1:
                                    nc.scalar.activation(d, s, copy_f, scale=sc)
                                )
                        st[ui]["w16"] = (td, ta)
                    else:
                        t16 = wd16_p.tile([128, 8192], f16)
                        sw = sc_sb[:, 8 + half : 9 + half]
                        use_dve = wd_eng == "dve" or (wd_eng == "alt" and half == 0)
                        for p in range(2):
                            dst = t16[:, p * 4096 : (p + 1) * 4096]
                            src = t8[:, p * 4096 : (p + 1) * 4096]
                            if not do_dequant:
                                dst, src = dst[:, ::32], src[:, ::32]
                            if use_dve:
                                thunks.append(
                                    lambda d=dst, s=src, sc=sw:
                                    nc.vector.tensor_scalar_mul(d, s, sc)
                                )
                            else:
                                thunks.append(
                                    lambda d=dst, s=src, sc=sw:
                                    nc.scalar.activation(d, s, copy_f, scale=sc)
                                )
                        st[ui]["w16"] = t16
                    return thunks

                def compute(ui, hook=None):
                    kind, j, half = units[ui]
                    if probe == "dma":
                        if kind == "d" and half == 1:
                            touch = sg_p.tile([128, 4, cap], f32, tag="touch")
                            nc.vector.tensor_copy(
                                touch[:, 0, :1], item_of(ui)["sc"][:, :1]
                            )
                            o_sb = o_p.tile([cap, 1024], f16)
                            nc.vector.tensor_copy(o_sb[:, :1], touch[:cap, 0, :1])
                            store_eng.dma_start(out=out_d[j], in_=o_sb[:])
                        return
                    if probe == "deqonly":
                        if kind == "d" and half == 1:
                            o_sb = o_p.tile([cap, 1024], f16)
                            nc.vector.tensor_copy(
                                o_sb[:], st[ui]["w16"][:cap, :1024]
                            )
                            store_eng.dma_start(out=out_d[j], in_=o_sb[:])
                        return
                    im = item_of(ui)
                    xt_sb, inter_sb = im["xt"], im["inter"]
                    if kind == "gu":
                        td, ta = st[ui]["w16"]
                        for gsub in range(2):
                            grp = half * 2 + gsub
                            # bank-padded psum tiles: 4 ht strips of 128 cols
                            ps_g = ps_gu.tile([128, 4, 128], f32, tag="ps_g")
                            ps_u = ps_gu.tile([128, 4, 128], f32, tag="ps_u")
                            for ht in range(4):
                                f0 = gsub * 1024 + (ht * 2) * 128
                                f1 = f0 + 128
                                for dc in range(8):
                                    w = td[:, dc] if dc < dve_dc else ta[:, dc - dve_dc]
                                    nc.tensor.matmul(
                                        ps_g[:, ht, :cap],
                                        w[:, f0 : f0 + 128],
                                        xt_sb[:, dc, :],
                                        start=(dc == 0),
                                        stop=(dc == 7),
                                    )
                                    nc.tensor.matmul(
                                        ps_u[:, ht, :cap],
                                        w[:, f1 : f1 + 128],
                                        xt_sb[:, dc, :],
                                        start=(dc == 0),
                                        stop=(dc == 7),
                                    )
                            sg = sg_p.tile([128, 4, cap], f32)
                            with tc.high_priority(offset=hp_epi) if hp_epi else _nullctx():
                                nc.scalar.activation(sg[:], ps_g[:, :, :cap], silu)
                                nc.vector.tensor_mul(
                                    inter_sb[:, grp * 4 : grp * 4 + 4, :],
                                    sg[:],
                                    ps_u[:, :, :cap],
                                )
                            if hook:
                                hook()
                    else:
                        if half == 0:
                            im["ps_o"] = ps_dn.tile([cap, 1024], f32, name="ps_o")
                        ps_o = im["ps_o"]
                        t16 = st[ui]["w16"]
                        for gsub in range(2):
                            for i in range(4):
                                hc = (half * 2 + gsub) * 4 + i
                                off = (gsub * 4 + i) * 1024
                                for nt in range(2):
                                    nc.tensor.matmul(
                                        ps_o[:, nt * 512 : (nt + 1) * 512],
                                        inter_sb[:, hc, :],
                                        t16[:, off + nt * 512 : off + (nt + 1) * 512],
                                        start=(hc == 0),
                                        stop=(hc == 15),
                                    )
                            if hook:
                                hook()
                        if half == 1:
                            o_sb = o_p.tile([cap, 1024], f16)
                            nc.vector.tensor_copy(o_sb[:], ps_o[:])
                            store_eng.dma_start(out=out_d[j], in_=o_sb[:])

                U = len(units)
                load(0)
                if U > 1:
                    load(1)
                for t in dequant_thunks(0):
                    t()
                for u in range(U):
                    if u + 2 < U:
                        load(u + 2)
                    tq = dequant_thunks(u + 1) if u + 1 < U else []
                    compute(u)
                    for t in tq:
                        t()

            if loop_n is None:
                emit_body()
            else:
                with tc.For_i(0, loop_n, 1):
                    emit_body()

    nc.compile()
    return nc


def _get_program(nw: int, cap: int):
    if (nw, cap) not in _CACHE:
        _CACHE[(nw, cap)] = _build(nw, cap)
    return _CACHE[(nw, cap)]


def _prepare(x, expert_indices, Wgu, Wd):
    """Host dispatch + quantization + layout rearrangement."""
    x = np.ascontiguousarray(np.asarray(x), dtype=np.float32)
    ei = np.asarray(expert_indices).astype(np.int64)
    Wgu = np.ascontiguousarray(np.asarray(Wgu), dtype=np.float32)
    Wd = np.ascontiguousarray(np.asarray(Wd), dtype=np.float32)

    # ---- host dispatch: group (t, k) slots by expert ----
    flat = ei.ravel()  # slot s = t*K + k
    order = np.argsort(flat, kind="stable")
    counts = np.bincount(flat, minlength=E)
    offs = np.concatenate(([0], np.cumsum(counts)))
    slots_e = [order[offs[e] : offs[e + 1]] for e in range(E)]

    # token capacity: pad the busiest expert block up to a multiple of 32
    cap = max(64, min(C, -(-int(counts.max()) // 32) * 32))

    # work items per core: (expert, token slots) with <= cap tokens each
    items = [[] for _ in range(NCORES)]
    for e in range(E):
        c = e // EPC
        s = slots_e[e]
        for b in range(max(1, -(-len(s) // cap))):
            items[c].append((e, s[b * cap : (b + 1) * cap]))
    nw = max(len(it) for it in items)
    for c in range(NCORES):
        while len(items[c]) < nw:
            items[c].append((c * EPC, np.empty(0, np.int64)))

    # ---- host quantization (int8 symmetric, per-channel) ----
    # Wgu: scale per (e, d) over the (x, h) axes.
    sgu = np.abs(Wgu).max(axis=(1, 2)) / 127.0            # (E, D)
    qgu = np.clip(np.round(Wgu / sgu[:, None, None, :]), -127, 127).astype(
        np.int8
    )                                                      # (E, 2, H, D)
    # Wd: scale per (e, half, hl) over (d, gsub, i).
    Wd_r = Wd.reshape(E, D, 2, 2, 4, 128)                  # e,d,half,gsub,i,hl
    swd = np.abs(Wd_r).max(axis=(1, 3, 4)) / 127.0         # (E, 2, 128)
    qd = np.clip(
        np.round(Wd_r / swd[:, None, :, None, None, :]), -127, 127
    ).astype(np.int8)                                      # (E,D,2,2,4,128)

    # ---- layout rearrangement (partition-major) ----
    # wgu8_all[e, half, dp, dc, gsub*1024 + (ht*2+g)*128 + hl]
    # from qgu[e, g, (half*2+gsub)*512 + ht*128 + hl, dc*128+dp]
    wgu8_all = (
        qgu.reshape(E, 2, 2, 2, 4, 128, 8, 128)            # e,g,half,gsub,ht,hl,dc,dp
        .transpose(0, 2, 7, 6, 3, 4, 1, 5)                 # e,half,dp,dc,gsub,ht,g,hl
        .reshape(E, 2, 128, 8, 2048)
    )
    # wd8_all[e, half, hl, (gsub*4+i)*1024+d] = qd[e, d, half, gsub, i, hl]
    wd8_all = np.ascontiguousarray(qd.transpose(0, 2, 5, 3, 4, 1)).reshape(
        E, 2, 128, 8192
    )
    # sc_all[e, p, 0:8] = sgu[e, dc*128+p]; sc_all[e, p, 8+half] = swd[e,half,p]
    sc_all = np.zeros((E, 128, 12), np.float32)
    sc_all[:, :, :8] = sgu.reshape(E, 8, 128).transpose(0, 2, 1)
    sc_all[:, :, 8:10] = swd.transpose(0, 2, 1)

    xf = x.astype(np.float16)

    in_maps = []
    for c in range(NCORES):
        xt_h = np.zeros((nw, 128, 8, cap), np.float16)
        eids = np.array([e for e, _ in items[c]])
        for idx, (e, slots) in enumerate(items[c]):
            n = len(slots)
            if n:
                blk = np.zeros((cap, D), np.float16)
                blk[:n] = xf[slots // K]
                xt_h[idx] = blk.T.reshape(8, 128, cap).transpose(1, 0, 2)
        in_maps.append(
            {
                "xt": xt_h,
                "wgu8": np.ascontiguousarray(wgu8_all[eids]),
                "wd8": np.ascontiguousarray(wd8_all[eids]),
                "sc": np.ascontiguousarray(sc_all[eids]),
            }
        )
    return in_maps, items, nw, cap


def _combine(results, items):
    out = np.zeros((T * K, D), np.float32)
    for c in range(NCORES):
        o_core = results[c]["out"]  # (nw, C, 1024) fp16
        for idx, (e, slots) in enumerate(items[c]):
            n = len(slots)
            if n:
                out[slots] = o_core[idx, :n].astype(np.float32)
    return out.reshape(T, K, D)


def kernel(x, expert_indices, Wgu, Wd):
    from concourse.bass_utils import run_bass_kernel_spmd

    in_maps, items, nw, cap = _prepare(x, expert_indices, Wgu, Wd)
    nc = _get_program(nw, cap)
    r = run_bass_kernel_spmd(nc, in_maps, list(range(NCORES)))
    kernel.last_results = r
    return _combine(r.results, items)

